# revision 2
# baseline (speedup 1.0000x reference)
"""Trainium2 Bass kernel for nn_HRNetW30classifier: logits = x @ W.T + b.

Shapes (full): x (8192, 2048) f32, W (1000, 2048) f32, b (1000,) f32
Output: (8192, 1000) f32.

Sharding: data-parallel over batch across 8 NeuronCores. Each core computes a
(1024, 2048) @ (2048, 1000) GEMM with W/b replicated.

Device kernel: host pre-transposes x and W so the contraction dim (K=2048)
lands on the SBUF partition axis (contiguous DMA rows) and casts to fp16. The
TensorEngine runs fp16 matmuls (1 col/cycle warm), accumulating fp32 in PSUM
over 16 K-tiles.

Schedule (timeline targets from NTFF trace analysis):
- ~7.2us framework preamble (fixed), then a short dummy-matmul burst covers
  the first-slice DMA latency and starts filling the HAM activity window so
  the PE clock ungates (1.2 -> 2.4 GHz) as early as possible.
- Input DMAs stream on the sync-engine HWDGE ring (qSPDynamicHW, FIFO) in
  need-order: w[kt0] n-chunk0 first, then the first two x m-tiles, bias row,
  the rest of kt0/kt1, then 2-kt-batched transfers (fewer 0.6us triggers).
- bias (1,1000) f32 is DMA'd as a single 4KB row and partition-broadcast
  on GpSimd during the ramp (saves 500KB of stream traffic).
- Phase 1: mt 0..3 k-outer, paced by the DMA stream (PE-bound once warm).
  A few dependency-free dummy matmuls are interleaved into the first kt
  group as insurance against stream jitter resetting the HAM window.
- Phase 2: mt 4..7 chunk-serial: 16-MM blocks per (mt, n-chunk), evicting
  each block while the next runs, so the final tail is one DVE add + one
  244KB DMA.
- Evictions (DVE bias-add + DMA out) go on the scalar-engine HWDGE ring
  (qActDynamicHW), fully decoupled from the input stream.
"""

import numpy as np

P = 128
N_CORES = 8
B_FULL = 8192
M = B_FULL // N_CORES  # 1024 batch rows per core
N = 1000  # classes
K = 2048  # features
KT = K // P  # 16 k-tiles
MT = M // P  # 8 m-tiles
MH = MT // 2  # 4 m-tiles per phase
N0_W = 512  # first n-chunk (one PSUM bank of fp32)
N1_W = N - N0_W  # 488

DUMMY_COLS = 64  # narrow warmup MMs: fine-grained PE busy filler (~29-55ns)
N_DUMMY = 22  # pre-real-MM warmup burst (~1.2us cold cover)
INS_DUMMIES = (6, 4, 2)  # insurance dummies after kt0 mt0/mt1/mt2 pairs

_NC_CACHE = {}


def _build_nc():
    """Build + compile the per-core Bass program (SPMD: same NEFF on 8 cores)."""
    from contextlib import ExitStack

    import concourse.tile as tile
    from concourse import bacc, mybir
    from concourse._compat import get_trn_type

    f32 = mybir.dt.float32
    f16 = mybir.dt.float16

    nc = bacc.Bacc(get_trn_type() or "TRN2", target_bir_lowering=False, debug=False)

    xT = nc.dram_tensor("xT", [K, M], f16, kind="ExternalInput")
    wT = nc.dram_tensor("wT", [K, N], f16, kind="ExternalInput")
    brow = nc.dram_tensor("brow", [1, N], f32, kind="ExternalInput")
    out = nc.dram_tensor("out", [M, N], f32, kind="ExternalOutput")

    xT_r = xT.ap().rearrange("(kt p) m -> kt p m", p=P)  # [KT, 128, M]
    xT_r2 = xT.ap().rearrange("(kb two p) m -> kb two p m", p=P, two=2)
    wT_r = wT.ap().rearrange("(kt p) n -> kt p n", p=P)  # [KT, 128, N]
    wT_r2 = wT.ap().rearrange("(kb two p) n -> kb two p n", p=P, two=2)
    out_r = out.ap().rearrange("(mt p) n -> mt p n", p=P)  # [MT, 128, N]

    with tile.TileContext(nc) as tc:
        with ExitStack() as ctx:
            xpool = ctx.enter_context(tc.tile_pool(name="xpool", bufs=1))
            wpool = ctx.enter_context(tc.tile_pool(name="wpool", bufs=1))
            bpool = ctx.enter_context(tc.tile_pool(name="bpool", bufs=1))
            opool = ctx.enter_context(tc.tile_pool(name="opool", bufs=10))
            pspool = ctx.enter_context(tc.tile_pool(name="ps", bufs=8, space="PSUM"))

            # Everything is resident in SBUF: x 32KB/part, W 31.25KB/part.
            x_sb = xpool.tile([P, KT, M], f16, tag="x")
            w_sb = wpool.tile([P, KT, N], f16, tag="w")
            wscr = bpool.tile([1, 256], f16, tag="wscr")
            b_sb = bpool.tile([1, N], f32, tag="brow")
            bias_t = bpool.tile([P, N], f32, tag="bias")

            # Input DMA stream on the sync HWDGE ring, in need-order. The
            # single FIFO queue completes transfers in issue order at full
            # HBM rate. First slices are split fine so the first real matmul
            # (x m-tile 0/1 + w kt0 n-chunk0) can start ~1us after the sync
            # engine's preamble ends; later kts are 2-kt batches (halves the
            # 0.6us/trigger sync-engine cost).
            nc.sync.dma_start(w_sb[:, 0, 0:N0_W], wT_r[0][:, 0:N0_W])
            nc.sync.dma_start(x_sb[:, 0, 0 : 2 * P], xT_r[0][:, 0 : 2 * P])
            nc.sync.dma_start(b_sb[:], brow.ap())
            nc.sync.dma_start(w_sb[:, 0, N0_W:N], wT_r[0][:, N0_W:N])
            nc.sync.dma_start(x_sb[:, 0, 2 * P : M], xT_r[0][:, 2 * P : M])
            nc.sync.dma_start(w_sb[:, 1, :], wT_r[1])
            nc.sync.dma_start(x_sb[:, 1, :], xT_r[1])
            for kb in range(1, KT // 2):
                nc.sync.dma_start(w_sb[:, 2 * kb : 2 * kb + 2, :], wT_r2[kb])
                nc.sync.dma_start(x_sb[:, 2 * kb : 2 * kb + 2, :], xT_r2[kb])

            # bias: broadcast the (1, N) row to all 128 partitions on GpSimd
            # during the ramp; needed only by the first eviction (~38us).
            nc.gpsimd.partition_broadcast(bias_t[:], b_sb[:])

            # Keep the PE busy through the HAM activity window with cheap
            # dummy matmuls on a dependency-free scratch tile, so the clock
            # gate reaches full rate (2.4GHz) as early as possible. These
            # begin the moment the PE preamble ends, covering the first
            # k-slice DMA wait.
            nc.gpsimd.memset(wscr[:], 1.0)
            ps_w = pspool.tile([P, N0_W], f32, tag="ps", name="ps_warm")

            def dummy(n=1):
                for _ in range(n):
                    nc.tensor.matmul(
                        ps_w[:, :DUMMY_COLS],
                        lhsT=wscr[:, 0:P],
                        rhs=wscr[:, 0:DUMMY_COLS],
                        start=True,
                        stop=True,
                    )

            dummy(N_DUMMY)

            def mm_pair(psA, psB, mt, kt, start, stop):
                lhsT = x_sb[:, kt, mt * P : (mt + 1) * P]
                nc.tensor.matmul(
                    psA[:, :N0_W],
                    lhsT=lhsT,
                    rhs=w_sb[:, kt, 0:N0_W],
                    start=start,
                    stop=stop,
                )
                nc.tensor.matmul(
                    psB[:, :N1_W],
                    lhsT=lhsT,
                    rhs=w_sb[:, kt, N0_W:N],
                    start=start,
                    stop=stop,
                )

            def evict(ps_t, mt, n0, nw):
                ot = opool.tile([P, N0_W], f32, tag="ot", name=f"ot_{n0}_{mt}")
                nc.vector.tensor_add(ot[:, :nw], ps_t[:, :nw], bias_t[:, n0 : n0 + nw])
                nc.scalar.dma_start(out_r[mt, :, n0 : n0 + nw], ot[:, :nw])

            def ps_pair(mt):
                a = pspool.tile([P, N0_W], f32, tag="ps", name=f"psA_{mt}")
                b = pspool.tile([P, N0_W], f32, tag="ps", name=f"psB_{mt}")
                return a, b

            # ---- phase 1: mt 0..3, k-outer, paced by the DMA stream ----
            ps1 = [ps_pair(mt) for mt in range(MH)]
            for kt in range(KT):
                for mt in range(MH):
                    mm_pair(*ps1[mt], mt, kt, start=(kt == 0), stop=(kt == KT - 1))
                    # Insurance dummies inside the first kt group: keep the
                    # PE busy if the stream ramp is a bit late (a single
                    # >~250ns gap resets the HAM busy window, costing ~3us).
                    # They must stay ahead of the mt3 B-chunk start (same
                    # PSUM bank as ps_w).
                    if kt == 0 and mt < len(INS_DUMMIES):
                        dummy(INS_DUMMIES[mt])
            for mt in range(MH):
                evict(ps1[mt][0], mt, 0, N0_W)
                evict(ps1[mt][1], mt, N0_W, N1_W)

            # ---- phase 2: mt 4..7, chunk-serial blocks; each block's
            # eviction overlaps the next block's matmuls ----
            for mt in range(MH, MT):
                a, b = ps_pair(mt)
                for kt in range(KT):
                    nc.tensor.matmul(
                        a[:, :N0_W],
                        lhsT=x_sb[:, kt, mt * P : (mt + 1) * P],
                        rhs=w_sb[:, kt, 0:N0_W],
                        start=(kt == 0),
                        stop=(kt == KT - 1),
                    )
                evict(a, mt, 0, N0_W)
                for kt in range(KT):
                    nc.tensor.matmul(
                        b[:, :N1_W],
                        lhsT=x_sb[:, kt, mt * P : (mt + 1) * P],
                        rhs=w_sb[:, kt, N0_W:N],
                        start=(kt == 0),
                        stop=(kt == KT - 1),
                    )
                evict(b, mt, N0_W, N1_W)

    nc.compile()
    return nc


def _get_nc():
    if "nc" not in _NC_CACHE:
        _NC_CACHE["nc"] = _build_nc()
    return _NC_CACHE["nc"]


def _run(in_maps, trace=False, **kwargs):
    from concourse.bass_utils import run_bass_kernel_spmd

    nc = _get_nc()
    return run_bass_kernel_spmd(
        nc, in_maps, core_ids=list(range(N_CORES)), trace=trace, **kwargs
    )


def _make_in_maps(x, W, b):
    x = np.asarray(x, dtype=np.float32)
    W = np.asarray(W, dtype=np.float32)
    b = np.asarray(b, dtype=np.float32)
    xT = np.ascontiguousarray(x.T).astype(np.float16)  # (K, B_FULL)
    wT = np.ascontiguousarray(W.T).astype(np.float16)  # (K, N)
    brow = np.ascontiguousarray(b[None, :])  # (1, N) f32
    return [
        {
            "xT": np.ascontiguousarray(xT[:, c * M : (c + 1) * M]),
            "wT": wT,
            "brow": brow,
        }
        for c in range(N_CORES)
    ]


def kernel(x, W, b):
    res = _run(_make_in_maps(x, W, b))
    return np.concatenate([r["out"] for r in res.results], axis=0)


# revision 6
# speedup vs baseline: 1.1661x; 1.1661x over previous
"""Trainium2 Bass kernel for nn_HRNetW30classifier: logits = x @ W.T + b.

Shapes (full): x (8192, 2048) f32, W (1000, 2048) f32, b (1000,) f32
Output: (8192, 1000) f32.

Sharding: data-parallel over batch across 8 NeuronCores. Each core computes a
(1024, 2048) @ (2048, 1000) GEMM with W/b replicated.

Device kernel: host pre-transposes x and W so the contraction dim (K=2048)
lands on the SBUF partition axis (contiguous DMA rows) and casts to fp16. The
TensorEngine runs fp16 matmuls (1 col/cycle warm), accumulating fp32 in PSUM
over 16 K-tiles.

Schedule (timeline targets from NTFF trace analysis):
- ~7.2us framework preamble (fixed), then a short dummy-matmul burst covers
  the first-slice DMA latency and starts filling the HAM activity window so
  the PE clock ungates (1.2 -> 2.4 GHz) as early as possible.
- Input DMAs stream on the sync-engine HWDGE ring (qSPDynamicHW, FIFO) in
  need-order: w[kt0] n-chunk0 first, then the first two x m-tiles, bias row,
  the rest of kt0/kt1, then 2-kt-batched transfers (fewer 0.6us triggers).
- bias (1,1000) f32 is DMA'd as a single 4KB row and partition-broadcast
  on GpSimd during the ramp (saves 500KB of stream traffic).
- Phase 1: mt 0..3 k-outer, paced by the DMA stream (PE-bound once warm).
  A few dependency-free dummy matmuls are interleaved into the first kt
  group as insurance against stream jitter resetting the HAM window.
- Phase 2: mt 4..7 chunk-serial: 16-MM blocks per (mt, n-chunk), evicting
  each block while the next runs, so the final tail is one DVE add + one
  244KB DMA.
- Evictions (DVE bias-add + DMA out) go on the scalar-engine HWDGE ring
  (qActDynamicHW), fully decoupled from the input stream.
"""

import numpy as np

P = 128
N_CORES = 8
B_FULL = 8192
M = B_FULL // N_CORES  # 1024 batch rows per core
N = 1000  # classes
K = 2048  # features
KT = K // P  # 16 k-tiles
MT = M // P  # 8 m-tiles
MH = MT // 2  # 4 m-tiles per phase
N0_W = 512  # first n-chunk (one PSUM bank of fp32)
N1_W = N - N0_W  # 488

DUMMY_COLS = 64  # narrow warmup MMs: fine-grained PE busy filler (~29-55ns)
N_DUMMY = 44  # pre-real-MM warmup burst (~2.5us cold cover to first-data)
INS_DUMMIES = (6, 4, 2)  # insurance dummies after kt0 mt0/mt1/mt2 pairs
N1A_W = 256  # mt7's B-chunk splits 488 -> 256 + 232 so the tail DMA is small

_NC_CACHE = {}


def _build_nc():
    """Build + compile the per-core Bass program (SPMD: same NEFF on 8 cores)."""
    from contextlib import ExitStack

    import concourse.tile as tile
    from concourse import bacc, mybir
    from concourse._compat import get_trn_type

    f32 = mybir.dt.float32
    f16 = mybir.dt.float16

    nc = bacc.Bacc(get_trn_type() or "TRN2", target_bir_lowering=False, debug=False)

    xT = nc.dram_tensor("xT", [K, M], f16, kind="ExternalInput")
    wT = nc.dram_tensor("wT", [K, N], f16, kind="ExternalInput")
    brow = nc.dram_tensor("brow", [1, N], f32, kind="ExternalInput")
    out = nc.dram_tensor("out", [M, N], f32, kind="ExternalOutput")

    xT_r = xT.ap().rearrange("(kt p) m -> kt p m", p=P)  # [KT, 128, M]
    wT_r = wT.ap().rearrange("(kt p) n -> kt p n", p=P)  # [KT, 128, N]
    out_r = out.ap().rearrange("(mt p) n -> mt p n", p=P)  # [MT, 128, N]

    with tile.TileContext(nc) as tc:
        with ExitStack() as ctx:
            xpool = ctx.enter_context(tc.tile_pool(name="xpool", bufs=1))
            wpool = ctx.enter_context(tc.tile_pool(name="wpool", bufs=1))
            bpool = ctx.enter_context(tc.tile_pool(name="bpool", bufs=1))
            opool = ctx.enter_context(tc.tile_pool(name="opool", bufs=10))
            pspool = ctx.enter_context(tc.tile_pool(name="ps", bufs=8, space="PSUM"))

            # Everything is resident in SBUF: x 32KB/part, W 31.25KB/part.
            x_sb = xpool.tile([P, KT, M], f16, tag="x")
            w_sb = wpool.tile([P, KT, N], f16, tag="w")
            wscr = bpool.tile([1, 256], f16, tag="wscr")
            b_sb = bpool.tile([1, N], f32, tag="brow")
            bias_t = bpool.tile([P, N], f32, tag="bias")

            # Input DMA stream in need-order across the two HWDGE rings.
            # Each trigger costs ~0.7us on its issuing engine and completion
            # (transfer + write receipt) lags the doorbell by ~1.5-2.5us, so
            # the very first slices (w kt0 n-chunk0 + x m-tiles 0-1) go on
            # the scalar ring, whose preamble ends ~1us before the sync
            # engine's -- evictions don't need that ring until ~40us. The
            # rest streams per-kt on the sync FIFO ring: finer granularity
            # keeps each kt's arrival just ahead of the PE's consumption.
            nc.scalar.dma_start(w_sb[:, 0, 0:N0_W], wT_r[0][:, 0:N0_W])
            nc.scalar.dma_start(x_sb[:, 0, 0 : 2 * P], xT_r[0][:, 0 : 2 * P])
            nc.sync.dma_start(w_sb[:, 0, N0_W:N], wT_r[0][:, N0_W:N])
            nc.sync.dma_start(x_sb[:, 0, 2 * P : M], xT_r[0][:, 2 * P : M])
            nc.sync.dma_start(w_sb[:, 1, :], wT_r[1])
            nc.sync.dma_start(x_sb[:, 1, :], xT_r[1])
            nc.sync.dma_start(b_sb[:], brow.ap())
            for kt in range(2, KT):
                nc.sync.dma_start(w_sb[:, kt, :], wT_r[kt])
                nc.sync.dma_start(x_sb[:, kt, :], xT_r[kt])

            # bias: broadcast the (1, N) row to all 128 partitions on GpSimd
            # during the ramp; needed only by the first eviction (~38us).
            nc.gpsimd.partition_broadcast(bias_t[:], b_sb[:])

            # Keep the PE busy through the HAM activity window with cheap
            # dummy matmuls on a dependency-free scratch tile, so the clock
            # gate reaches full rate (2.4GHz) as early as possible. These
            # begin the moment the PE preamble ends, covering the first
            # k-slice DMA wait.
            nc.gpsimd.memset(wscr[:], 1.0)
            ps_w = pspool.tile([P, N0_W], f32, tag="ps", name="ps_warm")

            def dummy(n=1):
                for _ in range(n):
                    nc.tensor.matmul(
                        ps_w[:, :DUMMY_COLS],
                        lhsT=wscr[:, 0:P],
                        rhs=wscr[:, 0:DUMMY_COLS],
                        start=True,
                        stop=True,
                    )

            dummy(N_DUMMY)

            def mm_pair(psA, psB, mt, kt, start, stop):
                lhsT = x_sb[:, kt, mt * P : (mt + 1) * P]
                nc.tensor.matmul(
                    psA[:, :N0_W],
                    lhsT=lhsT,
                    rhs=w_sb[:, kt, 0:N0_W],
                    start=start,
                    stop=stop,
                )
                nc.tensor.matmul(
                    psB[:, :N1_W],
                    lhsT=lhsT,
                    rhs=w_sb[:, kt, N0_W:N],
                    start=start,
                    stop=stop,
                )

            def evict(ps_t, mt, n0, nw):
                ot = opool.tile([P, N0_W], f32, tag="ot", name=f"ot_{n0}_{mt}")
                nc.vector.tensor_add(ot[:, :nw], ps_t[:, :nw], bias_t[:, n0 : n0 + nw])
                nc.scalar.dma_start(out_r[mt, :, n0 : n0 + nw], ot[:, :nw])

            def ps_pair(mt):
                a = pspool.tile([P, N0_W], f32, tag="ps", name=f"psA_{mt}")
                b = pspool.tile([P, N0_W], f32, tag="ps", name=f"psB_{mt}")
                return a, b

            # ---- phase 1: mt 0..3, k-outer, paced by the DMA stream ----
            ps1 = [ps_pair(mt) for mt in range(MH)]
            for kt in range(KT):
                for mt in range(MH):
                    mm_pair(*ps1[mt], mt, kt, start=(kt == 0), stop=(kt == KT - 1))
                    # Insurance dummies inside the first kt group: keep the
                    # PE busy if the stream ramp is a bit late (a single
                    # >~250ns gap resets the HAM busy window, costing ~3us).
                    # They must stay ahead of the mt3 B-chunk start (same
                    # PSUM bank as ps_w).
                    if kt == 0 and mt < len(INS_DUMMIES):
                        dummy(INS_DUMMIES[mt])
            for mt in range(MH):
                evict(ps1[mt][0], mt, 0, N0_W)
                evict(ps1[mt][1], mt, N0_W, N1_W)

            # ---- phase 2: mt 4..7, chunk-serial blocks; each block's
            # eviction overlaps the next block's matmuls. The very last mt
            # splits its B-chunk 488 -> 256+232 so the tail (one DVE add +
            # one DMA + completion receipt) is as small as possible. ----
            def block(ps_t, mt, n0, nw):
                for kt in range(KT):
                    nc.tensor.matmul(
                        ps_t[:, :nw],
                        lhsT=x_sb[:, kt, mt * P : (mt + 1) * P],
                        rhs=w_sb[:, kt, n0 : n0 + nw],
                        start=(kt == 0),
                        stop=(kt == KT - 1),
                    )
                evict(ps_t, mt, n0, nw)

            for mt in range(MH, MT):
                a, b = ps_pair(mt)
                block(a, mt, 0, N0_W)
                if mt == MT - 1:
                    c = pspool.tile([P, N0_W], f32, tag="ps", name="psC_tail")
                    block(b, mt, N0_W, N1A_W)
                    block(c, mt, N0_W + N1A_W, N1_W - N1A_W)
                else:
                    block(b, mt, N0_W, N1_W)

    nc.compile()
    return nc


def _get_nc():
    if "nc" not in _NC_CACHE:
        _NC_CACHE["nc"] = _build_nc()
    return _NC_CACHE["nc"]


def _run(in_maps, trace=False, **kwargs):
    from concourse.bass_utils import run_bass_kernel_spmd

    nc = _get_nc()
    return run_bass_kernel_spmd(
        nc, in_maps, core_ids=list(range(N_CORES)), trace=trace, **kwargs
    )


def _make_in_maps(x, W, b):
    x = np.asarray(x, dtype=np.float32)
    W = np.asarray(W, dtype=np.float32)
    b = np.asarray(b, dtype=np.float32)
    xT = np.ascontiguousarray(x.T).astype(np.float16)  # (K, B_FULL)
    wT = np.ascontiguousarray(W.T).astype(np.float16)  # (K, N)
    brow = np.ascontiguousarray(b[None, :])  # (1, N) f32
    return [
        {
            "xT": np.ascontiguousarray(xT[:, c * M : (c + 1) * M]),
            "wT": wT,
            "brow": brow,
        }
        for c in range(N_CORES)
    ]


def kernel(x, W, b):
    res = _run(_make_in_maps(x, W, b))
    return np.concatenate([r["out"] for r in res.results], axis=0)


# revision 8
# speedup vs baseline: 1.1799x; 1.0118x over previous
"""Trainium2 Bass kernel for nn_HRNetW30classifier: logits = x @ W.T + b.

Shapes (full): x (8192, 2048) f32, W (1000, 2048) f32, b (1000,) f32
Output: (8192, 1000) f32.

Sharding: data-parallel over batch across 8 NeuronCores. Each core computes a
(1024, 2048) @ (2048, 1000) GEMM with W/b replicated.

Device kernel: host pre-transposes x and W so the contraction dim (K=2048)
lands on the SBUF partition axis (contiguous DMA rows) and casts to fp16. The
TensorEngine runs fp16 matmuls (1 col/cycle warm), accumulating fp32 in PSUM
over 16 K-tiles.

Schedule (timeline targets from NTFF trace analysis):
- ~7.2us framework preamble (fixed), then a short dummy-matmul burst covers
  the first-slice DMA latency and starts filling the HAM activity window so
  the PE clock ungates (1.2 -> 2.4 GHz) as early as possible.
- Input DMAs stream on the sync-engine HWDGE ring (qSPDynamicHW, FIFO) in
  need-order: w[kt0] n-chunk0 first, then the first two x m-tiles, bias row,
  the rest of kt0/kt1, then 2-kt-batched transfers (fewer 0.6us triggers).
- bias (1,1000) f32 is DMA'd as a single 4KB row and partition-broadcast
  on GpSimd during the ramp (saves 500KB of stream traffic).
- Phase 1: mt 0..3 k-outer, paced by the DMA stream (PE-bound once warm).
  A few dependency-free dummy matmuls are interleaved into the first kt
  group as insurance against stream jitter resetting the HAM window.
- Phase 2: mt 4..7 chunk-serial: 16-MM blocks per (mt, n-chunk), evicting
  each block while the next runs, so the final tail is one DVE add + one
  244KB DMA.
- Evictions (DVE bias-add + DMA out) go on the scalar-engine HWDGE ring
  (qActDynamicHW), fully decoupled from the input stream.
"""

import numpy as np

P = 128
N_CORES = 8
B_FULL = 8192
M = B_FULL // N_CORES  # 1024 batch rows per core
N = 1000  # classes
K = 2048  # features
KT = K // P  # 16 k-tiles
MT = M // P  # 8 m-tiles
MH = MT // 2  # 4 m-tiles per phase
N0_W = 512  # first n-chunk (one PSUM bank of fp32)
N1_W = N - N0_W  # 488

DUMMY_COLS = 64  # narrow warmup MMs: fine-grained PE busy filler (~29-55ns)
N_DUMMY = 64  # pre-real-MM warmup burst: covers until first DMA completion
# (~11.5us: preamble ends ~7.3, first transfer completes ~11.3 -- the first
# dynamic DMA's doorbell->completion lag is ~3.5us on a cold DGE pipeline)
INS_DUMMIES = (6, 4, 2)  # insurance dummies after kt0 mt0/mt1/mt2 pairs
N1A_W = 256  # mt7's B-chunk splits 488 -> 256 + 232 so the tail DMA is small

_NC_CACHE = {}


def _build_nc():
    """Build + compile the per-core Bass program (SPMD: same NEFF on 8 cores)."""
    from contextlib import ExitStack

    import concourse.tile as tile
    from concourse import bacc, mybir
    from concourse._compat import get_trn_type

    f32 = mybir.dt.float32
    f16 = mybir.dt.float16

    nc = bacc.Bacc(get_trn_type() or "TRN2", target_bir_lowering=False, debug=False)

    xT = nc.dram_tensor("xT", [K, M], f16, kind="ExternalInput")
    wT = nc.dram_tensor("wT", [K, N], f16, kind="ExternalInput")
    brow = nc.dram_tensor("brow", [1, N], f32, kind="ExternalInput")
    out = nc.dram_tensor("out", [M, N], f32, kind="ExternalOutput")

    xT_r = xT.ap().rearrange("(kt p) m -> kt p m", p=P)  # [KT, 128, M]
    wT_r = wT.ap().rearrange("(kt p) n -> kt p n", p=P)  # [KT, 128, N]
    out_r = out.ap().rearrange("(mt p) n -> mt p n", p=P)  # [MT, 128, N]

    with tile.TileContext(nc) as tc:
        with ExitStack() as ctx:
            xpool = ctx.enter_context(tc.tile_pool(name="xpool", bufs=1))
            wpool = ctx.enter_context(tc.tile_pool(name="wpool", bufs=1))
            bpool = ctx.enter_context(tc.tile_pool(name="bpool", bufs=1))
            opool = ctx.enter_context(tc.tile_pool(name="opool", bufs=10))
            pspool = ctx.enter_context(tc.tile_pool(name="ps", bufs=8, space="PSUM"))

            # Everything is resident in SBUF: x 32KB/part, W 31.25KB/part.
            x_sb = xpool.tile([P, KT, M], f16, tag="x")
            w_sb = wpool.tile([P, KT, N], f16, tag="w")
            wscr = bpool.tile([1, 256], f16, tag="wscr")
            b_sb = bpool.tile([1, N], f32, tag="brow")
            bias_t = bpool.tile([P, N], f32, tag="bias")

            # Input DMA stream on the sync HWDGE ring, strict need-order.
            # The FIFO ring completes transfers in issue order; each trigger
            # costs ~0.65us on the sync engine and the first completion lags
            # its doorbell by ~3.5us (cold DGE pipeline), so the first real
            # matmul can start ~11.3us. After that the stream sustains
            # ~1.4us/kt-slice, ahead of the PE's 1.67us/kt consumption.
            nc.sync.dma_start(w_sb[:, 0, 0:N0_W], wT_r[0][:, 0:N0_W])
            nc.sync.dma_start(x_sb[:, 0, 0 : 2 * P], xT_r[0][:, 0 : 2 * P])
            nc.sync.dma_start(w_sb[:, 0, N0_W:N], wT_r[0][:, N0_W:N])
            nc.sync.dma_start(x_sb[:, 0, 2 * P : M], xT_r[0][:, 2 * P : M])
            nc.sync.dma_start(w_sb[:, 1, :], wT_r[1])
            nc.sync.dma_start(x_sb[:, 1, :], xT_r[1])
            nc.sync.dma_start(b_sb[:], brow.ap())
            for kt in range(2, KT):
                nc.sync.dma_start(w_sb[:, kt, :], wT_r[kt])
                nc.sync.dma_start(x_sb[:, kt, :], xT_r[kt])

            # bias: broadcast the (1, N) row to all 128 partitions on GpSimd
            # during the ramp; needed only by the first eviction (~38us).
            nc.gpsimd.partition_broadcast(bias_t[:], b_sb[:])

            # Keep the PE busy through the HAM activity window with cheap
            # dummy matmuls on a dependency-free scratch tile, so the clock
            # gate reaches full rate (2.4GHz) as early as possible. These
            # begin the moment the PE preamble ends, covering the first
            # k-slice DMA wait.
            nc.gpsimd.memset(wscr[:], 1.0)
            ps_w = pspool.tile([P, N0_W], f32, tag="ps", name="ps_warm")

            def dummy(n=1):
                for _ in range(n):
                    nc.tensor.matmul(
                        ps_w[:, :DUMMY_COLS],
                        lhsT=wscr[:, 0:P],
                        rhs=wscr[:, 0:DUMMY_COLS],
                        start=True,
                        stop=True,
                    )

            dummy(N_DUMMY)

            def mm_pair(psA, psB, mt, kt, start, stop):
                lhsT = x_sb[:, kt, mt * P : (mt + 1) * P]
                nc.tensor.matmul(
                    psA[:, :N0_W],
                    lhsT=lhsT,
                    rhs=w_sb[:, kt, 0:N0_W],
                    start=start,
                    stop=stop,
                )
                nc.tensor.matmul(
                    psB[:, :N1_W],
                    lhsT=lhsT,
                    rhs=w_sb[:, kt, N0_W:N],
                    start=start,
                    stop=stop,
                )

            def evict(ps_t, mt, n0, nw):
                ot = opool.tile([P, N0_W], f32, tag="ot", name=f"ot_{n0}_{mt}")
                nc.vector.tensor_add(ot[:, :nw], ps_t[:, :nw], bias_t[:, n0 : n0 + nw])
                nc.scalar.dma_start(out_r[mt, :, n0 : n0 + nw], ot[:, :nw])

            def ps_pair(mt):
                a = pspool.tile([P, N0_W], f32, tag="ps", name=f"psA_{mt}")
                b = pspool.tile([P, N0_W], f32, tag="ps", name=f"psB_{mt}")
                return a, b

            # ---- phase 1: mt 0..3, k-outer, paced by the DMA stream ----
            ps1 = [ps_pair(mt) for mt in range(MH)]
            for kt in range(KT):
                for mt in range(MH):
                    mm_pair(*ps1[mt], mt, kt, start=(kt == 0), stop=(kt == KT - 1))
                    # Insurance dummies inside the first kt group: keep the
                    # PE busy if the stream ramp is a bit late (a single
                    # >~250ns gap resets the HAM busy window, costing ~3us).
                    # They must stay ahead of the mt3 B-chunk start (same
                    # PSUM bank as ps_w).
                    if kt == 0 and mt < len(INS_DUMMIES):
                        dummy(INS_DUMMIES[mt])
            for mt in range(MH):
                evict(ps1[mt][0], mt, 0, N0_W)
                evict(ps1[mt][1], mt, N0_W, N1_W)

            # ---- phase 2: mt 4..7, chunk-serial blocks; each block's
            # eviction overlaps the next block's matmuls. The very last mt
            # splits its B-chunk 488 -> 256+232 so the tail (one DVE add +
            # one DMA + completion receipt) is as small as possible. ----
            def block(ps_t, mt, n0, nw):
                for kt in range(KT):
                    nc.tensor.matmul(
                        ps_t[:, :nw],
                        lhsT=x_sb[:, kt, mt * P : (mt + 1) * P],
                        rhs=w_sb[:, kt, n0 : n0 + nw],
                        start=(kt == 0),
                        stop=(kt == KT - 1),
                    )
                evict(ps_t, mt, n0, nw)

            for mt in range(MH, MT):
                a, b = ps_pair(mt)
                block(a, mt, 0, N0_W)
                if mt == MT - 1:
                    c = pspool.tile([P, N0_W], f32, tag="ps", name="psC_tail")
                    block(b, mt, N0_W, N1A_W)
                    block(c, mt, N0_W + N1A_W, N1_W - N1A_W)
                else:
                    block(b, mt, N0_W, N1_W)

    nc.compile()
    return nc


def _get_nc():
    if "nc" not in _NC_CACHE:
        _NC_CACHE["nc"] = _build_nc()
    return _NC_CACHE["nc"]


def _run(in_maps, trace=False, **kwargs):
    from concourse.bass_utils import run_bass_kernel_spmd

    nc = _get_nc()
    return run_bass_kernel_spmd(
        nc, in_maps, core_ids=list(range(N_CORES)), trace=trace, **kwargs
    )


def _make_in_maps(x, W, b):
    x = np.asarray(x, dtype=np.float32)
    W = np.asarray(W, dtype=np.float32)
    b = np.asarray(b, dtype=np.float32)
    xT = np.ascontiguousarray(x.T).astype(np.float16)  # (K, B_FULL)
    wT = np.ascontiguousarray(W.T).astype(np.float16)  # (K, N)
    brow = np.ascontiguousarray(b[None, :])  # (1, N) f32
    return [
        {
            "xT": np.ascontiguousarray(xT[:, c * M : (c + 1) * M]),
            "wT": wT,
            "brow": brow,
        }
        for c in range(N_CORES)
    ]


def kernel(x, W, b):
    res = _run(_make_in_maps(x, W, b))
    return np.concatenate([r["out"] for r in res.results], axis=0)


# revision 11
# speedup vs baseline: 1.1859x; 1.0051x over previous
"""Trainium2 Bass kernel for nn_HRNetW30classifier: logits = x @ W.T + b.

Shapes (full): x (8192, 2048) f32, W (1000, 2048) f32, b (1000,) f32
Output: (8192, 1000) f32.

Sharding: data-parallel over batch across 8 NeuronCores. Each core computes a
(1024, 2048) @ (2048, 1000) GEMM with W/b replicated.

Device kernel: host pre-transposes x and W so the contraction dim (K=2048)
lands on the SBUF partition axis (contiguous DMA rows) and casts to fp16. The
TensorEngine runs fp16 matmuls (1 col/cycle warm), accumulating fp32 in PSUM
over 16 K-tiles.

Schedule (timeline targets from NTFF trace analysis):
- ~7.2us framework preamble (fixed), then a short dummy-matmul burst covers
  the first-slice DMA latency and starts filling the HAM activity window so
  the PE clock ungates (1.2 -> 2.4 GHz) as early as possible.
- Input DMAs stream on the sync-engine HWDGE ring (qSPDynamicHW, FIFO) in
  need-order: w[kt0] n-chunk0 first, then the first two x m-tiles, bias row,
  the rest of kt0/kt1, then 2-kt-batched transfers (fewer 0.6us triggers).
- bias (1,1000) f32 is DMA'd as a single 4KB row and partition-broadcast
  on GpSimd during the ramp (saves 500KB of stream traffic).
- Phase 1: mt 0..3 k-outer, paced by the DMA stream (PE-bound once warm).
  A few dependency-free dummy matmuls are interleaved into the first kt
  group as insurance against stream jitter resetting the HAM window.
- Phase 2: mt 4..7 chunk-serial: 16-MM blocks per (mt, n-chunk), evicting
  each block while the next runs, so the final tail is one DVE add + one
  244KB DMA.
- Evictions (DVE bias-add + DMA out) go on the scalar-engine HWDGE ring
  (qActDynamicHW), fully decoupled from the input stream.
"""

import numpy as np

P = 128
N_CORES = 8
B_FULL = 8192
M = B_FULL // N_CORES  # 1024 batch rows per core
N = 1000  # classes
K = 2048  # features
KT = K // P  # 16 k-tiles
MT = M // P  # 8 m-tiles
MH = MT // 2  # 4 m-tiles per phase
N0_W = 512  # first n-chunk (one PSUM bank of fp32)
N1_W = N - N0_W  # 488

DUMMY_COLS = 64  # narrow warmup MMs: fine-grained PE busy filler (~29-55ns)
N_DUMMY = 56  # pre-real-MM warmup burst: covers until first DMA completion
# (~10.9us: preamble ends ~7.3, first transfer completes ~10.8 -- the first
# dynamic DMA's doorbell->completion lag is ~3us on a cold DGE pipeline)
INS_DUMMIES = (5, 4, 3, 2)  # insurance dummies after kt0 A-chunk MMs
TAIL_SPLIT = (256, 168, 64)  # mt7's B-chunk 488 -> small final eviction

_NC_CACHE = {}


def _build_nc():
    """Build + compile the per-core Bass program (SPMD: same NEFF on 8 cores)."""
    from contextlib import ExitStack

    import concourse.tile as tile
    from concourse import bacc, mybir
    from concourse._compat import get_trn_type

    f32 = mybir.dt.float32
    f16 = mybir.dt.float16

    nc = bacc.Bacc(get_trn_type() or "TRN2", target_bir_lowering=False, debug=False)

    xT = nc.dram_tensor("xT", [K, M], f16, kind="ExternalInput")
    wT = nc.dram_tensor("wT", [K, N], f16, kind="ExternalInput")
    brow = nc.dram_tensor("brow", [1, N], f32, kind="ExternalInput")
    out = nc.dram_tensor("out", [M, N], f32, kind="ExternalOutput")

    xT_r = xT.ap().rearrange("(kt p) m -> kt p m", p=P)  # [KT, 128, M]
    wT_r = wT.ap().rearrange("(kt p) n -> kt p n", p=P)  # [KT, 128, N]
    out_r = out.ap().rearrange("(mt p) n -> mt p n", p=P)  # [MT, 128, N]

    with tile.TileContext(nc) as tc:
        with ExitStack() as ctx:
            xpool = ctx.enter_context(tc.tile_pool(name="xpool", bufs=1))
            wpool = ctx.enter_context(tc.tile_pool(name="wpool", bufs=1))
            bpool = ctx.enter_context(tc.tile_pool(name="bpool", bufs=1))
            opool = ctx.enter_context(tc.tile_pool(name="opool", bufs=10))
            pspool = ctx.enter_context(tc.tile_pool(name="ps", bufs=8, space="PSUM"))

            # Everything is resident in SBUF: x 32KB/part, W 31.25KB/part.
            x_sb = xpool.tile([P, KT, M], f16, tag="x")
            w_sb = wpool.tile([P, KT, N], f16, tag="w")
            wscr = bpool.tile([1, 256], f16, tag="wscr")
            b_sb = bpool.tile([1, N], f32, tag="brow")
            bias_t = bpool.tile([P, N], f32, tag="bias")

            # Input DMA stream on the sync HWDGE ring, strict need-order.
            # The FIFO ring completes transfers in issue order; each trigger
            # costs ~0.65us on the sync engine and the first completion lags
            # its doorbell by ~3.5us (cold DGE pipeline), so the first real
            # matmul can start ~11.3us. After that the stream sustains
            # ~1.4us/kt-slice, ahead of the PE's 1.67us/kt consumption.
            nc.sync.dma_start(w_sb[:, 0, 0:N0_W], wT_r[0][:, 0:N0_W])
            nc.sync.dma_start(x_sb[:, 0, 0 : 2 * P], xT_r[0][:, 0 : 2 * P])
            nc.sync.dma_start(w_sb[:, 0, N0_W:N], wT_r[0][:, N0_W:N])
            nc.sync.dma_start(x_sb[:, 0, 2 * P : M], xT_r[0][:, 2 * P : M])
            nc.sync.dma_start(w_sb[:, 1, :], wT_r[1])
            nc.sync.dma_start(x_sb[:, 1, :], xT_r[1])
            nc.sync.dma_start(b_sb[:], brow.ap())
            for kt in range(2, KT):
                nc.sync.dma_start(w_sb[:, kt, :], wT_r[kt])
                nc.sync.dma_start(x_sb[:, kt, :], xT_r[kt])

            # bias: broadcast the (1, N) row to all 128 partitions on GpSimd
            # during the ramp; needed only by the first eviction (~38us).
            nc.gpsimd.partition_broadcast(bias_t[:], b_sb[:])

            # Keep the PE busy through the HAM activity window with cheap
            # dummy matmuls on a dependency-free scratch tile, so the clock
            # gate reaches full rate (2.4GHz) as early as possible. These
            # begin the moment the PE preamble ends, covering the first
            # k-slice DMA wait.
            nc.gpsimd.memset(wscr[:], 1.0)
            ps_w = pspool.tile([P, N0_W], f32, tag="ps", name="ps_warm")

            def dummy(n=1):
                for _ in range(n):
                    nc.tensor.matmul(
                        ps_w[:, :DUMMY_COLS],
                        lhsT=wscr[:, 0:P],
                        rhs=wscr[:, 0:DUMMY_COLS],
                        start=True,
                        stop=True,
                    )

            dummy(N_DUMMY)

            def mm_pair(psA, psB, mt, kt, start, stop):
                lhsT = x_sb[:, kt, mt * P : (mt + 1) * P]
                nc.tensor.matmul(
                    psA[:, :N0_W],
                    lhsT=lhsT,
                    rhs=w_sb[:, kt, 0:N0_W],
                    start=start,
                    stop=stop,
                )
                nc.tensor.matmul(
                    psB[:, :N1_W],
                    lhsT=lhsT,
                    rhs=w_sb[:, kt, N0_W:N],
                    start=start,
                    stop=stop,
                )

            def evict(ps_t, mt, n0, nw):
                ot = opool.tile([P, N0_W], f32, tag="ot", name=f"ot_{n0}_{mt}")
                nc.vector.tensor_add(ot[:, :nw], ps_t[:, :nw], bias_t[:, n0 : n0 + nw])
                nc.scalar.dma_start(out_r[mt, :, n0 : n0 + nw], ot[:, :nw])

            def ps_pair(mt):
                a = pspool.tile([P, N0_W], f32, tag="ps", name=f"psA_{mt}")
                b = pspool.tile([P, N0_W], f32, tag="ps", name=f"psB_{mt}")
                return a, b

            # ---- phase 1: mt 0..3, k-outer, paced by the DMA stream ----
            # kt0 runs all four A-chunk MMs first: they need only w0 n-chunk0
            # and the x m-tiles (which land in stream order), so real work
            # starts as soon as the first two transfers complete, with no
            # dependency on w0 n-chunk1. Insurance dummies woven between
            # them keep the PE busy if the stream ramp is late (a single
            # >~250ns gap resets the HAM busy window, costing ~3us of cold
            # clock). All dummies stay ahead of the mt3 B-chunk start (same
            # PSUM bank as ps_w).
            ps1 = [ps_pair(mt) for mt in range(MH)]
            for mt in range(MH):
                nc.tensor.matmul(
                    ps1[mt][0][:, :N0_W],
                    lhsT=x_sb[:, 0, mt * P : (mt + 1) * P],
                    rhs=w_sb[:, 0, 0:N0_W],
                    start=True,
                    stop=False,
                )
                dummy(INS_DUMMIES[mt])
            for mt in range(MH):
                nc.tensor.matmul(
                    ps1[mt][1][:, :N1_W],
                    lhsT=x_sb[:, 0, mt * P : (mt + 1) * P],
                    rhs=w_sb[:, 0, N0_W:N],
                    start=True,
                    stop=False,
                )
            for kt in range(1, KT):
                for mt in range(MH):
                    mm_pair(*ps1[mt], mt, kt, start=False, stop=(kt == KT - 1))
            for mt in range(MH):
                evict(ps1[mt][0], mt, 0, N0_W)
                evict(ps1[mt][1], mt, N0_W, N1_W)

            # ---- phase 2: mt 4..7, chunk-serial blocks; each block's
            # eviction overlaps the next block's matmuls. The very last mt
            # splits its B-chunk 488 -> 256+232 so the tail (one DVE add +
            # one DMA + completion receipt) is as small as possible. ----
            def block(ps_t, mt, n0, nw):
                for kt in range(KT):
                    nc.tensor.matmul(
                        ps_t[:, :nw],
                        lhsT=x_sb[:, kt, mt * P : (mt + 1) * P],
                        rhs=w_sb[:, kt, n0 : n0 + nw],
                        start=(kt == 0),
                        stop=(kt == KT - 1),
                    )
                evict(ps_t, mt, n0, nw)

            for mt in range(MH, MT):
                a, b = ps_pair(mt)
                block(a, mt, 0, N0_W)
                if mt == MT - 1:
                    n0 = N0_W
                    for i, nw in enumerate(TAIL_SPLIT):
                        t = b if i == 0 else pspool.tile(
                            [P, N0_W], f32, tag="ps", name=f"psT_{i}"
                        )
                        block(t, mt, n0, nw)
                        n0 += nw
                else:
                    block(b, mt, N0_W, N1_W)

    nc.compile()
    return nc


def _get_nc():
    if "nc" not in _NC_CACHE:
        _NC_CACHE["nc"] = _build_nc()
    return _NC_CACHE["nc"]


def _run(in_maps, trace=False, **kwargs):
    from concourse.bass_utils import run_bass_kernel_spmd

    nc = _get_nc()
    return run_bass_kernel_spmd(
        nc, in_maps, core_ids=list(range(N_CORES)), trace=trace, **kwargs
    )


def _make_in_maps(x, W, b):
    x = np.asarray(x, dtype=np.float32)
    W = np.asarray(W, dtype=np.float32)
    b = np.asarray(b, dtype=np.float32)
    xT = np.ascontiguousarray(x.T).astype(np.float16)  # (K, B_FULL)
    wT = np.ascontiguousarray(W.T).astype(np.float16)  # (K, N)
    brow = np.ascontiguousarray(b[None, :])  # (1, N) f32
    return [
        {
            "xT": np.ascontiguousarray(xT[:, c * M : (c + 1) * M]),
            "wT": wT,
            "brow": brow,
        }
        for c in range(N_CORES)
    ]


def kernel(x, W, b):
    res = _run(_make_in_maps(x, W, b))
    return np.concatenate([r["out"] for r in res.results], axis=0)
